# revision 10
# baseline (speedup 1.0000x reference)
"""Trainium2 Bass kernel for nn_AnimaPHCorrected (dense MoE with Boltzmann
gate, camp split, PH correction).

Strategy: data-parallel over the batch — each of the 8 NeuronCores gets
B/8 = 512 rows and evaluates all 8 experts locally (dense MoE), so no
collectives are needed.  Per core:

  gate:  scores = x @ gate_w / e       (fp32 PE matmuls, N=8)
         softmax -> top-5 mask -> renormalised weights   (DVE/ACT)
  L1:    hT[dh, b] = relu(w1.T x.T + b1)   (float32r matmuls, hT layout
         keeps D_H on partitions so L2 contracts naturally)
  L2:    e_out[b, o] = hT.T @ w2           (float32r matmuls)
         camp accumulators += weight[b,e] * e_out   (DVE scalar_tensor_tensor)
  PH:    diff = out_a - out_g; l2/var/sigmoid correction; out = diff*2*corr

float32r (tf32-like storage-fp32 PE mode) runs at bf16 rate (1 cyc/row,
measured ~1.5e-4 rel err for K=1024) vs true fp32's 4 cyc/row.

Weights are pre-tiled on the host so every DMA is a large per-partition-
contiguous transfer.
"""

import os
import sys

if "/opt/trn_rl_repo" not in sys.path:
    sys.path.insert(0, "/opt/trn_rl_repo")

import numpy as np

import concourse.bacc as bacc
import concourse.mybir as mybir
import concourse.tile as tile
from concourse import bass_utils
from concourse.masks import make_identity

P = 128
B = 4096
D_IN = 1024
D_H = 4096
D_OUT = 1024
E = 8
N_CORES = 8
B_LOC = B // N_CORES          # 512 rows per core
BM = B_LOC // P               # 4 partition tiles of local batch
KI = D_IN // P                # 8 k-tiles for layer 1
KH = D_H // P                 # 32 k-tiles for layer 2
MH = D_H // P                 # 32 m-tiles of D_H in layer 1
NO = D_OUT // 512             # 2 n-tiles of D_OUT in layer 2
KB = 4                        # k-tiles per w2 DMA block
CAP = 360                     # sparse capacity per (core, expert)
CT = (CAP + P - 1) // P
N_ACTIVE = 5
TEMP = float(np.e)
N_CAMP_A = E // 2

F32 = mybir.dt.float32
F32R = mybir.dt.float32r
BF16 = mybir.dt.bfloat16

# Results of the last device run (test harness reads exec_time_ns etc).
LAST_RESULTS = None
_NC_CACHE = {}


def _build(ph_alpha: float, ph_beta: float):
    """Build the per-core Bass program (SPMD: same program on all cores)."""
    nc = bacc.Bacc("TRN2", target_bir_lowering=False, debug=False)

    xt = nc.declare_dram_parameter("xt", [D_IN, B_LOC], F32, isOutput=False)
    gw = nc.declare_dram_parameter("gw", [D_IN, E], F32, isOutput=False)
    b1t = nc.declare_dram_parameter("b1t", [P, E, MH], F32, isOutput=False)
    w1t = nc.declare_dram_parameter(
        "w1t", [E, MH, P, KI, P], BF16, isOutput=False
    )
    w2t = nc.declare_dram_parameter(
        "w2t", [E, NO, KH // KB, P, KB, 512], BF16, isOutput=False
    )
    out = nc.declare_dram_parameter("out", [B_LOC, D_OUT], F32, isOutput=True)
    outa = nc.declare_dram_parameter("outa", [B_LOC, D_OUT], F32, isOutput=True)
    outg = nc.declare_dram_parameter("outg", [B_LOC, D_OUT], F32, isOutput=True)

    AL = mybir.AluOpType
    AF = mybir.ActivationFunctionType

    with tile.TileContext(nc) as tc:
        with (
            tc.tile_pool(name="big", bufs=1) as big,
            tc.tile_pool(name="wpool", bufs=10) as wpool,
            tc.tile_pool(name="small", bufs=2) as small,
            tc.tile_pool(name="wts", bufs=BM) as wtspool,
            tc.tile_pool(name="psum1", bufs=3, space="PSUM") as psum1,
            tc.tile_pool(name="psum2", bufs=4, space="PSUM") as psum2,
        ):
            # ---- static loads ----
            xt_f32 = big.tile([P, KI, B_LOC], F32, tag="xt")
            nc.sync.dma_start(xt_f32[:], xt[:].rearrange("(ko p) b -> p ko b", p=P))
            gwt = big.tile([P, KI, E], F32, tag="gw")
            nc.sync.dma_start(gwt[:], gw[:].rearrange("(ko p) e -> p ko e", p=P))
            b1s = big.tile([P, E, MH], F32, tag="b1")
            nc.sync.dma_start(b1s[:], b1t[:])

            x_r = big.tile([P, KI, B_LOC], BF16, tag="xr")
            nc.vector.tensor_copy(out=x_r[:], in_=xt_f32[:])

            # ---- gate: softmax over E, top-5 mask, renorm ----
            wts = []
            for bm in range(BM):
                psg = psum1.tile([P, E], F32, tag="ps1")
                for k in range(KI):
                    nc.tensor.matmul(
                        psg[:],
                        lhsT=xt_f32[:, k, bm * P : (bm + 1) * P],
                        rhs=gwt[:, k, :],
                        start=(k == 0),
                        stop=(k == KI - 1),
                    )
                sc = small.tile([P, E], F32, tag="sc")
                nc.vector.tensor_scalar_mul(sc[:], psg[:], 1.0 / TEMP)
                mx = small.tile([P, 1], F32, tag="mx")
                nc.vector.reduce_max(mx[:], sc[:], axis=mybir.AxisListType.X)
                nmx = small.tile([P, 1], F32, tag="nmx")
                nc.vector.tensor_scalar_mul(nmx[:], mx[:], -1.0)
                ex = small.tile([P, E], F32, tag="ex")
                se = small.tile([P, 1], F32, tag="se")
                nc.scalar.activation(
                    ex[:], sc[:], AF.Exp, bias=nmx[:], scale=1.0, accum_out=se[:]
                )
                rse = small.tile([P, 1], F32, tag="rse")
                nc.vector.reciprocal(rse[:], se[:])
                probs = small.tile([P, E], F32, tag="probs")
                nc.vector.tensor_scalar_mul(probs[:], ex[:], rse[:])

                work = small.tile([P, E], F32, tag="work")
                nc.vector.tensor_copy(out=work[:], in_=probs[:])
                sel = small.tile([P, E], F32, tag="sel")
                nc.vector.memset(sel[:], 0.0)
                for _ in range(N_ACTIVE):
                    m = small.tile([P, 1], F32, tag="m")
                    nc.vector.reduce_max(m[:], work[:], axis=mybir.AxisListType.X)
                    eq = small.tile([P, E], F32, tag="eq")
                    nc.vector.tensor_scalar(
                        out=eq[:], in0=work[:], scalar1=m[:], scalar2=None,
                        op0=AL.is_equal,
                    )
                    nc.vector.tensor_add(sel[:], sel[:], eq[:])
                    # work -= 1e30 * eq  (knock out the selected entry)
                    nc.vector.scalar_tensor_tensor(
                        out=work[:], in0=eq[:], scalar=-1e30, in1=work[:],
                        op0=AL.mult, op1=AL.add,
                    )
                wsel = small.tile([P, E], F32, tag="wsel")
                nc.vector.tensor_mul(wsel[:], probs[:], sel[:])
                ssum = small.tile([P, 1], F32, tag="ssum")
                nc.vector.reduce_sum(ssum[:], wsel[:], axis=mybir.AxisListType.X)
                nc.vector.tensor_scalar_add(ssum[:], ssum[:], 1e-8)
                rws = small.tile([P, 1], F32, tag="rws")
                nc.vector.reciprocal(rws[:], ssum[:])
                wv = wtspool.tile([P, E], F32, tag="wts")
                nc.vector.tensor_scalar_mul(wv[:], wsel[:], rws[:])
                wts.append(wv)

            # ---- camp accumulators ----
            acc_a = big.tile([P, BM, D_OUT], F32, tag="acca")
            nc.vector.memset(acc_a[:], 0.0)
            acc_g = big.tile([P, BM, D_OUT], F32, tag="accg")
            nc.vector.memset(acc_g[:], 0.0)

            # ---- expert loop ----
            for e in range(E):
                acc = acc_a if e < N_CAMP_A else acc_g

                # L1: hT[dh_tile, b] = relu(w1.T @ xT + b1)
                ht = big.tile([P, MH, B_LOC], BF16, tag="ht")
                for m in range(MH):
                    w1tile = wpool.tile([P, KI, P], BF16, tag="w1")
                    nc.sync.dma_start(w1tile[:], w1t[e, m])
                    ps = psum1.tile([P, B_LOC], F32, tag="ps1")
                    for k in range(KI):
                        nc.tensor.matmul(
                            ps[:],
                            lhsT=w1tile[:, k, :],
                            rhs=x_r[:, k, :],
                            start=(k == 0),
                            stop=(k == KI - 1),
                        )
                    nc.scalar.activation(
                        ht[:, m, :], ps[:], AF.Relu,
                        bias=b1s[:, e, m : m + 1], scale=1.0,
                    )

                # L2: e_out[b, o] accumulated over D_H; weighted into camps
                for n in range(NO):
                    ps2 = [
                        psum2.tile([P, 512], F32, tag="ps2", name=f"ps2_{bm}")
                        for bm in range(BM)
                    ]
                    for kb in range(KH // KB):
                        w2tile = wpool.tile([P, KB, 512], BF16, tag="w2")
                        nc.sync.dma_start(w2tile[:], w2t[e, n, kb])
                        for k4 in range(KB):
                            k = kb * KB + k4
                            for bm in range(BM):
                                nc.tensor.matmul(
                                    ps2[bm][:],
                                    lhsT=ht[:, k, bm * P : (bm + 1) * P],
                                    rhs=w2tile[:, k4, :],
                                    start=(k == 0),
                                    stop=(k == KH - 1),
                                )
                    for bm in range(BM):
                        # acc += wts[bm][:, e] * e_out
                        nc.vector.scalar_tensor_tensor(
                            out=acc[:, bm, n * 512 : (n + 1) * 512],
                            in0=ps2[bm][:],
                            scalar=wts[bm][:, e : e + 1],
                            in1=acc[:, bm, n * 512 : (n + 1) * 512],
                            op0=AL.mult,
                            op1=AL.add,
                        )

            # ---- PH correction + outputs ----
            for bm in range(BM):
                diff = small.tile([P, D_OUT], F32, tag="diff")
                nc.vector.tensor_sub(diff[:], acc_a[:, bm, :], acc_g[:, bm, :])
                sq = small.tile([P, D_OUT], F32, tag="sq")
                ssq = small.tile([P, 1], F32, tag="ssq")
                nc.scalar.activation(
                    sq[:], diff[:], AF.Square, scale=1.0, accum_out=ssq[:]
                )
                dsum = small.tile([P, 1], F32, tag="dsum")
                nc.vector.reduce_sum(dsum[:], diff[:], axis=mybir.AxisListType.X)
                l2 = small.tile([P, 1], F32, tag="l2")
                nc.scalar.activation(l2[:], ssq[:], AF.Sqrt)
                # var = ssq/D - (dsum/D)^2
                m1 = small.tile([P, 1], F32, tag="m1")
                nc.vector.tensor_scalar_mul(m1[:], dsum[:], 1.0 / D_OUT)
                m2 = small.tile([P, 1], F32, tag="m2")
                nc.vector.tensor_mul(m2[:], m1[:], m1[:])
                var = small.tile([P, 1], F32, tag="var")
                nc.vector.scalar_tensor_tensor(
                    out=var[:], in0=ssq[:], scalar=1.0 / D_OUT, in1=m2[:],
                    op0=AL.mult, op1=AL.subtract,
                )
                onepv = small.tile([P, 1], F32, tag="onepv")
                nc.vector.tensor_scalar_add(onepv[:], var[:], 1.0)
                ph = small.tile([P, 1], F32, tag="ph")
                nc.vector.tensor_mul(ph[:], l2[:], onepv[:])
                corr = small.tile([P, 1], F32, tag="corr")
                nc.scalar.activation(
                    corr[:], ph[:], AF.Sigmoid, scale=float(ph_alpha),
                    bias=float(ph_beta),
                )
                outt = small.tile([P, D_OUT], F32, tag="outt")
                nc.vector.tensor_scalar(
                    out=outt[:], in0=diff[:], scalar1=corr[:], scalar2=2.0,
                    op0=AL.mult, op1=AL.mult,
                )
                nc.sync.dma_start(out[bm * P : (bm + 1) * P, :], outt[:])
                nc.sync.dma_start(outa[bm * P : (bm + 1) * P, :], acc_a[:, bm, :])
                nc.sync.dma_start(outg[bm * P : (bm + 1) * P, :], acc_g[:, bm, :])

    nc.finalize()
    return nc


def build_sparse(ph_alpha: float, ph_beta: float):
    nc = bacc.Bacc("TRN2", target_bir_lowering=False, debug=False)

    xt = nc.declare_dram_parameter("xt", [D_IN, B_LOC], F32, isOutput=False)
    xr = nc.declare_dram_parameter("xr", [B_LOC, D_IN], BF16, isOutput=False)
    gw = nc.declare_dram_parameter("gw", [D_IN, E], F32, isOutput=False)
    b1t = nc.declare_dram_parameter("b1t", [P, E, MH], F32, isOutput=False)
    w1t = nc.declare_dram_parameter("w1t", [E, MH, P, KI, P], BF16, isOutput=False)
    w2t = nc.declare_dram_parameter(
        "w2t", [E, NO, KH // KB, P, KB, 512], BF16, isOutput=False
    )
    out = nc.declare_dram_parameter("out", [B_LOC, D_OUT], F32, isOutput=True)
    outa = nc.declare_dram_parameter("outa", [B_LOC, D_OUT], F32, isOutput=True)
    outg = nc.declare_dram_parameter("outg", [B_LOC, D_OUT], F32, isOutput=True)

    AL = mybir.AluOpType
    AF = mybir.ActivationFunctionType

    with tile.TileContext(nc) as tc:
        with (
            tc.tile_pool(name="big", bufs=1) as big,
            tc.tile_pool(name="wpool", bufs=10) as wpool,
            tc.tile_pool(name="w2pool", bufs=7) as w2pool,
            tc.tile_pool(name="small", bufs=2) as small,
            tc.tile_pool(name="gate", bufs=1) as gate,
            tc.tile_pool(name="wts", bufs=BM) as wtspool,
            tc.tile_pool(name="route", bufs=2) as route,
            tc.tile_pool(name="psum1", bufs=3, space="PSUM") as psum1,
            tc.tile_pool(name="psum2", bufs=4, space="PSUM") as psum2,
            tc.tile_pool(name="dram", bufs=1, space="DRAM") as dram,
        ):
            # ---- static loads / constants ----
            xt_f32 = big.tile([P, KI, B_LOC], F32, tag="xt")
            for bm in range(BM):
                nc.sync.dma_start(
                    xt_f32[:, :, bm * P : (bm + 1) * P],
                    xt[:, bm * P : (bm + 1) * P].rearrange(
                        "(ko p) b -> p ko b", p=P
                    ),
                )
            gwt = big.tile([P, KI, E], F32, tag="gw")
            nc.sync.dma_start(gwt[:], gw[:].rearrange("(ko p) e -> p ko e", p=P))
            xrow = big.tile([P, BM, D_IN], BF16, tag="xrow")
            nc.sync.dma_start(xrow[:], xr[:].rearrange("(rt p) d -> p rt d", p=P))
            b1s = big.tile([P, E, MH], F32, tag="b1")
            nc.sync.dma_start(b1s[:], b1t[:])

            ident = big.tile([P, P], F32, tag="ident")
            make_identity(nc, ident[:])
            iota_f = big.tile([P, CAP], F32, tag="iota_f")
            nc.gpsimd.iota(
                iota_f[:], pattern=[[1, CAP]], base=0, channel_multiplier=0,
                allow_small_or_imprecise_dtypes=True,
            )
            iota_offs = []
            for ct in range(CT):
                io = big.tile([P, 1], F32, tag=f"ioff{ct}", name=f"ioff{ct}")
                nc.gpsimd.iota(
                    io[:], pattern=[[1, 1]], base=ct * P, channel_multiplier=1,
                    allow_small_or_imprecise_dtypes=True,
                )
                iota_offs.append(io)

            # ---- gate (fp32): softmax over E, top-5, renorm ----
            # all 4 row-tiles batched as [128, 4, 8]; per-(p,bm) scalars are
            # applied via free-dim-broadcast tensor_tensor ops
            sc32 = gate.tile([P, BM, E], F32, tag="sc32")
            for bm in range(BM):
                psg = psum1.tile([P, E], F32, tag="ps1", name=f"psg{bm}")
                for k in range(KI):
                    nc.tensor.matmul(
                        psg[:],
                        lhsT=xt_f32[:, k, bm * P : (bm + 1) * P],
                        rhs=gwt[:, k, :],
                        start=(k == 0),
                        stop=(k == KI - 1),
                    )
                nc.vector.tensor_scalar_mul(sc32[:, bm, :], psg[:], 1.0 / TEMP)
            mx = gate.tile([P, BM], F32, tag="mx")
            nc.vector.reduce_max(mx[:], sc32[:], axis=mybir.AxisListType.X)
            ex32 = gate.tile([P, BM, E], F32, tag="ex32")
            nc.vector.tensor_sub(
                ex32[:], sc32[:], mx[:, :, None].to_broadcast([P, BM, E])
            )
            nc.scalar.activation(ex32[:], ex32[:], AF.Exp)
            se = gate.tile([P, BM], F32, tag="se")
            nc.vector.reduce_sum(se[:], ex32[:], axis=mybir.AxisListType.X)
            rse = gate.tile([P, BM], F32, tag="rse")
            nc.vector.reciprocal(rse[:], se[:])
            probs = gate.tile([P, BM, E], F32, tag="probs")
            nc.vector.tensor_mul(
                probs[:], ex32[:], rse[:, :, None].to_broadcast([P, BM, E])
            )
            # top-5 = knock out the bottom 3, then keep work < 1e29
            work = gate.tile([P, BM, E], F32, tag="work")
            nc.vector.tensor_copy(out=work[:], in_=probs[:])
            for _ in range(E - N_ACTIVE):
                mn = gate.tile([P, BM], F32, tag="mn")
                nc.vector.tensor_reduce(
                    mn[:], work[:], axis=mybir.AxisListType.X, op=AL.min
                )
                eq = gate.tile([P, BM, E], F32, tag="eq")
                nc.vector.tensor_tensor(
                    eq[:], work[:], mn[:, :, None].to_broadcast([P, BM, E]),
                    AL.is_equal,
                )
                nc.vector.scalar_tensor_tensor(
                    out=work[:], in0=eq[:], scalar=1e30, in1=work[:],
                    op0=AL.mult, op1=AL.add,
                )
            sel = gate.tile([P, BM, E], F32, tag="sel")
            nc.vector.tensor_scalar(
                out=sel[:], in0=work[:], scalar1=1e29, scalar2=None, op0=AL.is_lt
            )
            wsel = gate.tile([P, BM, E], F32, tag="wsel")
            nc.vector.tensor_mul(wsel[:], probs[:], sel[:])
            ssum = gate.tile([P, BM], F32, tag="ssum")
            nc.vector.reduce_sum(ssum[:], wsel[:], axis=mybir.AxisListType.X)
            nc.vector.tensor_scalar_add(ssum[:], ssum[:], 1e-8)
            rws = gate.tile([P, BM], F32, tag="rws")
            nc.vector.reciprocal(rws[:], ssum[:])
            wv32 = wtspool.tile([P, BM, E], F32, tag="wts")
            nc.vector.tensor_mul(
                wv32[:], wsel[:], rws[:, :, None].to_broadcast([P, BM, E])
            )
            wts = [wv32[:, bm, :] for bm in range(BM)]

            # ---- routing tables ----
            wtT = big.tile([8, B_LOC], F32, tag="wtT")
            for rt in range(BM):
                pt = psum1.tile([P, P], F32, tag="ps1", name=f"ptw{rt}")
                nc.tensor.transpose(pt[:8, :], wts[rt], ident[:])
                nc.vector.tensor_copy(out=wtT[:, rt * P : (rt + 1) * P], in_=pt[:8, :])
            mT = big.tile([8, B_LOC], F32, tag="mT")
            nc.vector.tensor_scalar(
                out=mT[:], in0=wtT[:], scalar1=0.0, scalar2=None, op0=AL.is_gt
            )
            cs = big.tile([8, B_LOC], F32, tag="cs")
            nc.vector.tensor_tensor_scan(
                out=cs[:], data0=mT[:], data1=mT[:], initial=0.0,
                op0=AL.add, op1=AL.bypass,
            )
            sT = big.tile([8, B_LOC], F32, tag="sT")
            nc.vector.tensor_mul(sT[:], cs[:], mT[:])
            nc.vector.tensor_scalar_add(sT[:], sT[:], -1.0)
            slot_row = big.tile([P, BM, 8], F32, tag="slot_row")
            for rt in range(BM):
                pt2 = psum1.tile([P, 8], F32, tag="ps1", name=f"pts{rt}")
                nc.tensor.transpose(
                    pt2[:], sT[:, rt * P : (rt + 1) * P], ident[:8, :8]
                )
                nc.vector.tensor_copy(out=slot_row[:, rt, :], in_=pt2[:])
            rt_dram = dram.tile([2, 8, B_LOC], F32, tag="rt_dram")
            nc.sync.dma_start(rt_dram[0], sT[:])
            nc.sync.dma_start(rt_dram[1], wtT[:])

            # ---- camp accumulators ----
            acc_a = big.tile([P, BM, D_OUT], F32, tag="acca")
            nc.vector.memset(acc_a[:], 0.0)
            acc_g = big.tile([P, BM, D_OUT], F32, tag="accg")
            nc.vector.memset(acc_g[:], 0.0)

            # ---- expert loop ----
            for e in range(E):
                acc = acc_a if e < N_CAMP_A else acc_g

                sb_b = route.tile([P, B_LOC], F32, tag="sb_b")
                nc.sync.dma_start(sb_b[:], rt_dram[0, e].partition_broadcast(P))
                wb_b = route.tile([P, B_LOC], F32, tag="wb_b")
                nc.sync.dma_start(wb_b[:], rt_dram[1, e].partition_broadcast(P))

                pe = route.tile([P, BM, CAP], BF16, tag="pe")
                for rt in range(BM):
                    nc.vector.tensor_scalar(
                        out=pe[:, rt, :], in0=iota_f[:],
                        scalar1=slot_row[:, rt, e : e + 1], scalar2=None,
                        op0=AL.is_equal,
                    )
                peT = route.tile([P, CT, B_LOC], F32R, tag="peT")
                for ct in range(CT):
                    nc.vector.scalar_tensor_tensor(
                        out=peT[:, ct, :], in0=sb_b[:], scalar=iota_offs[ct][:],
                        in1=wb_b[:], op0=AL.is_equal, op1=AL.mult,
                    )

                # gather: xg[d, c] (bf16) = sum_r x[r, d] P_e[r, c]
                xg = big.tile([P, KI, CAP], BF16, tag="xg")
                for dt in range(KI):
                    pg = psum1.tile([P, CAP], F32, tag="ps1", name=f"pg{e}_{dt}")
                    for rt in range(BM):
                        nc.tensor.matmul(
                            pg[:],
                            lhsT=xrow[:, rt, dt * P : (dt + 1) * P],
                            rhs=pe[:, rt, :],
                            start=(rt == 0),
                            stop=(rt == BM - 1),
                        )
                    nc.scalar.activation(xg[:, dt, :], pg[:], AF.Copy)

                # L1: hgt = relu(w1^T xg + b1)   [128, 32, CAP] bf16
                hgt = big.tile([P, MH, CAP], BF16, tag="hgt")
                for m in range(MH):
                    w1tile = wpool.tile([P, KI, P], BF16, tag="w1")
                    nc.sync.dma_start(w1tile[:], w1t[e, m])
                    ps = psum1.tile([P, CAP], F32, tag="ps1", name=f"ps1_{e}_{m}")
                    for k in range(KI):
                        nc.tensor.matmul(
                            ps[:],
                            lhsT=w1tile[:, k, :],
                            rhs=xg[:, k, :],
                            start=(k == 0),
                            stop=(k == KI - 1),
                        )
                    nc.scalar.activation(
                        hgt[:, m, :], ps[:], AF.Relu,
                        bias=b1s[:, e, m : m + 1], scale=1.0,
                    )

                # L2: ce[c, o] = hgt^T w2   -> f32r SBUF
                ce = big.tile([P, CT, NO, 512], F32R, tag="ce")
                for n in range(NO):
                    ps2 = [
                        psum2.tile([P, 512], F32, tag="ps2", name=f"ps2_{e}_{n}_{ct}")
                        for ct in range(CT)
                    ]
                    for kb in range(KH // KB):
                        w2tile = w2pool.tile([P, KB, 512], BF16, tag="w2")
                        nc.sync.dma_start(w2tile[:], w2t[e, n, kb])
                        for k4 in range(KB):
                            k = kb * KB + k4
                            for ct in range(CT):
                                cw = min(P, CAP - ct * P)
                                nc.tensor.matmul(
                                    ps2[ct][:cw],
                                    lhsT=hgt[:, k, ct * P : ct * P + cw],
                                    rhs=w2tile[:, k4, :],
                                    start=(k == 0),
                                    stop=(k == KH - 1),
                                )
                    for ct in range(CT):
                        cw = min(P, CAP - ct * P)
                        nc.scalar.activation(
                            ce[:cw, ct, n, :], ps2[ct][:cw], AF.Copy
                        )

                # scatter: acc[r, o] += sum_c P_eT[c, r] ce[c, o]
                for rt in range(BM):
                    for n in range(NO):
                        psc = psum1.tile(
                            [P, 512], F32, tag="ps1", name=f"psc{e}_{rt}_{n}"
                        )
                        for ct in range(CT):
                            cw = min(P, CAP - ct * P)
                            nc.tensor.matmul(
                                psc[:],
                                lhsT=peT[:cw, ct, rt * P : (rt + 1) * P],
                                rhs=ce[:cw, ct, n, :],
                                start=(ct == 0),
                                stop=(ct == CT - 1),
                            )
                        nc.vector.tensor_add(
                            acc[:, rt, n * 512 : (n + 1) * 512],
                            acc[:, rt, n * 512 : (n + 1) * 512],
                            psc[:],
                        )

                if e == N_CAMP_A - 1:
                    for bm in range(BM):
                        nc.sync.dma_start(
                            outa[bm * P : (bm + 1) * P, :], acc_a[:, bm, :]
                        )

            # ---- PH correction + outputs ----
            for bm in range(BM):
                diff = small.tile([P, D_OUT], F32, tag="diff")
                nc.vector.tensor_sub(diff[:], acc_a[:, bm, :], acc_g[:, bm, :])
                sq = small.tile([P, D_OUT], F32, tag="sq")
                ssq = small.tile([P, 1], F32, tag="ssq")
                nc.scalar.activation(
                    sq[:], diff[:], AF.Square, scale=1.0, accum_out=ssq[:]
                )
                dsum = small.tile([P, 1], F32, tag="dsum")
                nc.vector.reduce_sum(dsum[:], diff[:], axis=mybir.AxisListType.X)
                l2 = small.tile([P, 1], F32, tag="l2")
                nc.scalar.activation(l2[:], ssq[:], AF.Sqrt)
                m1 = small.tile([P, 1], F32, tag="m1")
                nc.vector.tensor_scalar_mul(m1[:], dsum[:], 1.0 / D_OUT)
                m2 = small.tile([P, 1], F32, tag="m2")
                nc.vector.tensor_mul(m2[:], m1[:], m1[:])
                var = small.tile([P, 1], F32, tag="var")
                nc.vector.scalar_tensor_tensor(
                    out=var[:], in0=ssq[:], scalar=1.0 / D_OUT, in1=m2[:],
                    op0=AL.mult, op1=AL.subtract,
                )
                onepv = small.tile([P, 1], F32, tag="onepv")
                nc.vector.tensor_scalar_add(onepv[:], var[:], 1.0)
                ph = small.tile([P, 1], F32, tag="ph")
                nc.vector.tensor_mul(ph[:], l2[:], onepv[:])
                corr = small.tile([P, 1], F32, tag="corr")
                nc.scalar.activation(
                    corr[:], ph[:], AF.Sigmoid, scale=float(ph_alpha),
                    bias=float(ph_beta),
                )
                outt = small.tile([P, D_OUT], F32, tag="outt")
                nc.vector.tensor_scalar(
                    out=outt[:], in0=diff[:], scalar1=corr[:], scalar2=2.0,
                    op0=AL.mult, op1=AL.mult,
                )
                nc.sync.dma_start(out[bm * P : (bm + 1) * P, :], outt[:])
                nc.sync.dma_start(outg[bm * P : (bm + 1) * P, :], acc_g[:, bm, :])

    nc.finalize()
    return nc


def _get_nc(ph_alpha: float, ph_beta: float, variant: str):
    key = (round(float(ph_alpha), 9), round(float(ph_beta), 9), variant)
    if key not in _NC_CACHE:
        builder = build_sparse if variant == "sparse" else _build
        _NC_CACHE[key] = builder(key[0], key[1])
    return _NC_CACHE[key]


def _routing_counts_ok(x, gate_w):
    """Host check that every (core, expert) routed count fits the sparse
    capacity (with margin for device/host fp32 tie differences)."""
    scores = (x @ gate_w) / TEMP
    s = scores - scores.max(axis=-1, keepdims=True)
    p = np.exp(s)
    p /= p.sum(axis=-1, keepdims=True)
    kth = np.partition(p, E - N_ACTIVE, axis=-1)[:, E - N_ACTIVE : E - N_ACTIVE + 1]
    mask = p >= kth
    counts = mask.reshape(N_CORES, B_LOC, E).sum(axis=1)
    return counts.max() <= CAP - 8


def _reference_numpy(x, gate_w, gate_b, w1, b1, w2, b2, ph_alpha, ph_beta):
    """Pure-numpy fallback (only used if inputs deviate from the fixed
    problem instance, e.g. nonzero gate_b/b2)."""
    scores = (x @ gate_w + gate_b) / TEMP
    scores = scores - scores.max(axis=-1, keepdims=True)
    probs = np.exp(scores)
    probs /= probs.sum(axis=-1, keepdims=True)
    idx = np.argsort(-probs, axis=-1, kind="stable")[:, :N_ACTIVE]
    mask = np.zeros_like(probs)
    np.put_along_axis(mask, idx, 1.0, axis=-1)
    w = probs * mask
    weights = w / (w.sum(axis=-1, keepdims=True) + 1e-8)
    h = np.maximum(np.einsum("bi,eih->beh", x, w1) + b1, 0.0)
    e_out = np.einsum("beh,eho->beo", h, w2) + b2
    out_a = np.einsum("be,beo->bo", weights[:, :N_CAMP_A], e_out[:, :N_CAMP_A])
    out_g = np.einsum("be,beo->bo", weights[:, N_CAMP_A:], e_out[:, N_CAMP_A:])
    repulsion = out_a - out_g
    l2 = np.linalg.norm(repulsion, axis=-1)
    var = np.var(repulsion, axis=-1)
    ph_dist = l2 * (1.0 + var)
    ph_corr = 2.0 / (1.0 + np.exp(-(ph_alpha * ph_dist + ph_beta)))
    output = repulsion * ph_corr[:, None]
    return (
        output.astype(np.float32),
        out_a.astype(np.float32),
        out_g.astype(np.float32),
    )


def kernel(x, gate_w, gate_b, w1, b1, w2, b2, ph_alpha, ph_beta):
    global LAST_RESULTS
    x = np.asarray(x, np.float32)
    gate_w = np.asarray(gate_w, np.float32)
    gate_b = np.asarray(gate_b, np.float32)
    w1 = np.asarray(w1, np.float32)
    b1 = np.asarray(b1, np.float32)
    w2 = np.asarray(w2, np.float32)
    b2 = np.asarray(b2, np.float32)
    alpha = float(np.asarray(ph_alpha))
    beta = float(np.asarray(ph_beta))

    if (
        x.shape != (B, D_IN)
        or w1.shape != (E, D_IN, D_H)
        or w2.shape != (E, D_H, D_OUT)
        or np.any(gate_b)
        or np.any(b2)
    ):
        # the device program folds gate_b/b2 out (they are zero in this
        # problem instance); anything else goes through numpy
        return _reference_numpy(
            x, gate_w, gate_b, w1, b1, w2, b2, alpha, beta
        )

    use_sparse = _routing_counts_ok(x, gate_w)
    nc = _get_nc(alpha, beta, "sparse" if use_sparse else "dense")

    # host pre-tiling (shared across cores)
    import ml_dtypes

    w1t = np.ascontiguousarray(
        w1.reshape(E, KI, P, MH, P).transpose(0, 3, 2, 1, 4)
    ).astype(ml_dtypes.bfloat16)
    w2t = np.ascontiguousarray(
        w2.reshape(E, KH // KB, KB, P, NO, 512).transpose(0, 4, 1, 3, 2, 5)
    ).astype(ml_dtypes.bfloat16)
    b1t = np.ascontiguousarray(b1.reshape(E, MH, P).transpose(2, 0, 1))
    gw = np.ascontiguousarray(gate_w)

    in_maps = []
    for c in range(N_CORES):
        xs = x[c * B_LOC : (c + 1) * B_LOC]
        m = {
            "xt": np.ascontiguousarray(xs.T),
            "gw": gw,
            "b1t": b1t,
            "w1t": w1t,
            "w2t": w2t,
        }
        if use_sparse:
            m["xr"] = np.ascontiguousarray(xs).astype(ml_dtypes.bfloat16)
        in_maps.append(m)

    res = bass_utils.run_bass_kernel_spmd(
        nc, in_maps, core_ids=list(range(N_CORES))
    )
    LAST_RESULTS = res

    output = np.concatenate([res.results[c]["out"] for c in range(N_CORES)], axis=0)
    out_a = np.concatenate([res.results[c]["outa"] for c in range(N_CORES)], axis=0)
    out_g = np.concatenate([res.results[c]["outg"] for c in range(N_CORES)], axis=0)
    return output, out_a, out_g
